# revision 2
# baseline (speedup 1.0000x reference)
"""Trainium2 Bass kernel for nn_Decoder_45380624450003.

Multi-head attention decoder + single-head pointer attention, data-parallel
over the batch dim across 8 NeuronCores (8 batches per core).

v3: global wave pipeline across all batches. Per global wave w the emission
order is [exp(w-1), ST(w), AV(w-2)] so the ACT engine never waits on a
just-signalled semaphore and the PE stream stays dense across batch
boundaries. k/q/v projection PSUM lives in the pa/pb single-bank "highway"
(the 6-bank ST ring stays pure so its 2-buffer parity holds). Weight DMAs
ride the idle GpSimd queue; input DMAs keep the SP queue. Final softmax
normalize runs on GpSimd (normalize_recip). Engine warmup releases the
PE/ACT activity throttles during the initial DMA wait.
"""

import sys

sys.path.insert(0, "/opt/trn_rl_repo")

from contextlib import ExitStack

import numpy as np

import concourse.bacc as bacc
import concourse.tile as tile
from concourse import mybir

F32 = mybir.dt.float32
FP16 = mybir.dt.float16
AF = mybir.ActivationFunctionType

B, N, E, H, D = 64, 500, 128, 8, 16
NCORES = 8
BPC = B // NCORES  # batches per core
NCH = 4
CH = N // NCH  # 125 rows per n/m chunk
SQRT_EMB = 11.313708498984761
CLIP = 10.0
COLW = 512  # psum bank width in f32
WAVES = [(0, 3), (3, 6), (6, 8)]  # head ranges per ST/exp wave
WPB = len(WAVES) * NCH  # waves per batch (12)


def _emit(ctx, tc, ap, probs, bpc, with_mask, use_pool_nr=True):
    nc = tc.nc

    const = ctx.enter_context(tc.tile_pool(name="const", bufs=1))
    io = ctx.enter_context(tc.tile_pool(name="io", bufs=4))
    proj = ctx.enter_context(tc.tile_pool(name="proj", bufs=2))
    etp = ctx.enter_context(tc.tile_pool(name="et", bufs=1))
    work = ctx.enter_context(tc.tile_pool(name="work", bufs=2))
    outp = ctx.enter_context(tc.tile_pool(name="outp", bufs=3))
    stp = ctx.enter_context(tc.tile_pool(name="st", bufs=2, space="PSUM"))
    pap = ctx.enter_context(tc.tile_pool(name="pa", bufs=1, space="PSUM"))
    pbp = ctx.enter_context(tc.tile_pool(name="pb", bufs=1, space="PSUM"))

    # ---- input DMAs for batch 0 first (SP queue), weights on GpSimd queue --
    def emit_dma_in(b):
        xf = io.tile([E, N], FP16, tag="xf", name="xf")
        q1 = io.tile([E, N], FP16, tag="q1", name="q1")
        q0 = io.tile([E, N], FP16, tag="q0", name="q0")
        nc.sync.dma_start(out=xf, in_=ap["xfT"][b])
        nc.sync.dma_start(out=q1, in_=ap["q1T"][b])
        nc.sync.dma_start(out=q0, in_=ap["q0T"][b])
        mT = mN = None
        if with_mask:
            mT = [io.tile([CH, N], F32, tag=f"mT{i}", name=f"mT{i}") for i in range(NCH)]
            mN = [io.tile([CH, N], F32, tag=f"mN{i}", name=f"mN{i}") for i in range(NCH)]
            for mc in range(NCH):
                nc.sync.dma_start(
                    out=mT[mc], in_=ap["maskT"][b, mc * CH : (mc + 1) * CH, :]
                )
                nc.sync.dma_start(
                    out=mN[mc], in_=ap["maskN"][b, mc * CH : (mc + 1) * CH, :]
                )
        return {"xf": xf, "q1": q1, "q0": q0, "mT": mT, "mN": mN}

    iot0 = emit_dma_in(0)

    w = {}
    for k in ["WkA", "WkB", "Wq1A", "Wq1B", "Wq0A", "Wq0B", "Wv", "WcA", "WcB",
              "MEXP"]:
        w[k] = const.tile([E, E], FP16, tag=k, name=k)
        nc.gpsimd.dma_start(out=w[k], in_=ap[k])

    iot1 = emit_dma_in(1) if bpc > 1 else None

    # ---- engine warmup: release PE/ACT activity throttles during DMA wait --
    warm = const.tile([E, COLW], FP16, tag="warm", name="warm")
    nc.vector.memset(warm[:, :], 0.125)
    wsb = const.tile([E, 2048], FP16, tag="wsb", name="wsb")
    nc.vector.memset(wsb[:, :], 0.25)
    for i in range(10):
        pool = pap if i % 2 == 0 else pbp
        tag = "pa" if i % 2 == 0 else "pb"
        wp = pool.tile([128, COLW], F32, tag=tag, name="warmps")
        nc.tensor.matmul(
            wp[:, 0:N], warm[:, 0:128], warm[:, 0:N], start=True, stop=True
        )
    nc.scalar.activation(out=wsb[:, 0:500], in_=wsb[:, 0:500], func=AF.Tanh)
    for i in range(2):
        nc.scalar.activation(out=wsb[:, 0:2000], in_=wsb[:, 0:2000], func=AF.Exp)

    def emit_kq_proj(iot):
        """k/q projections in the pa/pb highway + casts to fp16 SBUF."""
        xf, q1, q0 = iot["xf"], iot["q1"], iot["q0"]
        kpsA = pap.tile([128, COLW], F32, tag="pa", name="kpsA")
        kpsB = pbp.tile([128, COLW], F32, tag="pb", name="kpsB")
        nc.tensor.matmul(kpsA[:, 0:N], w["WkA"], xf, start=True, stop=True)
        nc.tensor.matmul(kpsB[:, 0:N], w["WkB"], xf, start=True, stop=True)
        kTa = proj.tile([E, N], FP16, tag="kTa", name="kTa")
        kTb = proj.tile([E, N], FP16, tag="kTb", name="kTb")
        nc.vector.tensor_copy(out=kTa, in_=kpsA[:, 0:N])
        nc.vector.tensor_copy(out=kTb, in_=kpsB[:, 0:N])

        qpsA = pap.tile([128, COLW], F32, tag="pa", name="qpsA")
        qpsB = pbp.tile([128, COLW], F32, tag="pb", name="qpsB")
        nc.tensor.matmul(qpsA[:, 0:N], w["Wq1A"], q1, start=True, stop=False)
        nc.tensor.matmul(qpsA[:, 0:N], w["Wq0A"], q0, start=False, stop=True)
        nc.tensor.matmul(qpsB[:, 0:N], w["Wq1B"], q1, start=True, stop=False)
        nc.tensor.matmul(qpsB[:, 0:N], w["Wq0B"], q0, start=False, stop=True)
        qTa = proj.tile([E, N], FP16, tag="qTa", name="qTa")
        qTb = proj.tile([E, N], FP16, tag="qTb", name="qTb")
        nc.vector.tensor_copy(out=qTa, in_=qpsA[:, 0:N])
        nc.vector.tensor_copy(out=qTb, in_=qpsB[:, 0:N])
        return {"kTa": kTa, "kTb": kTb, "qTa": qTa, "qTb": qTb}

    def emit_v_proj(iot):
        """v projection: vps fits one PSUM bank (4 chunks x 128 cols)."""
        xf = iot["xf"]
        vps = pap.tile([128, COLW], F32, tag="pa", name="vps")
        for mc in range(NCH):
            nc.tensor.matmul(
                vps[0:CH, mc * E : (mc + 1) * E],
                xf[:, mc * CH : (mc + 1) * CH],
                w["Wv"],
                start=True,
                stop=True,
            )
        va = [
            proj.tile([CH, H * 32], FP16, tag=f"va{mc}", name=f"va{mc}")
            for mc in range(NCH)
        ]
        for mc in range(NCH):
            var = va[mc].rearrange("p (h c) -> p h c", h=H)
            nc.vector.memset(va[mc][:, :], 0.0)
            nc.vector.memset(var[:, :, 16:17], 1.0)
            nc.vector.tensor_copy(
                out=var[:, :, 0:16],
                in_=vps[0:CH, mc * E : (mc + 1) * E].rearrange(
                    "p (h d) -> p h d", h=H
                ),
            )
        return va

    def emit_norm(bs):
        """Merged normalization: expander matmul -> reciprocal -> normalize
        -> combine (bias folded into WcA row 16 via the ones row of ota)."""
        usa, usb = bs["usa"], bs["usb"]
        rsA = pap.tile([128, COLW], F32, tag="pa", name="rsA")
        rsB = pbp.tile([128, COLW], F32, tag="pb", name="rsB")
        nc.tensor.matmul(rsA[:, 0:N], w["MEXP"], usa, start=True, stop=True)
        nc.tensor.matmul(rsB[:, 0:N], w["MEXP"], usb, start=True, stop=True)
        recA = work.tile([E, N], F32, tag="recA", name="recA")
        recB = work.tile([E, N], F32, tag="recB", name="recB")
        with nc.allow_low_precision(reason="fp16 softmax normalize"):
            nc.vector.reciprocal_approx_fast(out=recA, in_=rsA[:, 0:N])
            nc.vector.reciprocal_approx_fast(out=recB, in_=rsB[:, 0:N])
            ota = work.tile([E, N], FP16, tag="ota", name="ota")
            otb = work.tile([E, N], FP16, tag="otb", name="otb")
            nc.vector.tensor_mul(out=ota, in0=usa, in1=recA)
            nc.vector.tensor_mul(out=otb, in0=usb, in1=recB)
        mps = pap.tile([128, COLW], F32, tag="pa", name="mps")
        nc.tensor.matmul(mps[:, 0:N], w["WcA"], ota, start=True, stop=False)
        nc.tensor.matmul(mps[:, 0:N], w["WcB"], otb, start=False, stop=True)
        mh = work.tile([E, N], FP16, tag="mh", name="mh")
        with nc.allow_low_precision(reason="fp16 mh for pointer matmul"):
            nc.vector.tensor_copy(out=mh, in_=mps[:, 0:N])
        bs["mh"] = mh

    def emit_final_chunk(bs, c):
        """Pointer attention for chunk c of a finished batch."""
        mh, xf_, mN_ = bs["mh"], bs["iot"]["xf"], bs["iot"]["mN"]
        b_ = bs["b"]
        pool = pap if c % 2 == 0 else pbp
        tag = "pa" if c % 2 == 0 else "pb"
        sps = pool.tile([128, COLW], F32, tag=tag, name="sps")
        nc.tensor.matmul(
            sps[0:CH, 0:N],
            mh[:, c * CH : (c + 1) * CH],
            xf_,
            start=True,
            stop=True,
        )
        th = outp.tile([CH, N], F32, tag="th", name="th")
        nc.scalar.activation(
            out=th, in_=sps[0:CH, 0:N], func=AF.Tanh, scale=1.0 / SQRT_EMB
        )
        e2 = outp.tile([CH, N], F32, tag="e2", name="e2")
        sm = outp.tile([CH, 1], F32, tag="sm", name="sm")
        if with_mask:
            tm = outp.tile([CH, N], F32, tag="tm", name="tm")
            nc.vector.scalar_tensor_tensor(
                out=tm,
                in0=th,
                scalar=CLIP,
                in1=mN_[c],
                op0=mybir.AluOpType.mult,
                op1=mybir.AluOpType.add,
            )
            nc.scalar.activation(out=e2, in_=tm, func=AF.Exp, accum_out=sm)
        else:
            nc.scalar.activation(
                out=e2, in_=th, func=AF.Exp, scale=CLIP, accum_out=sm
            )
        pr = outp.tile([CH, N], F32, tag="pr", name="pr")
        if use_pool_nr:
            nc.gpsimd.normalize_recip(out_ap=pr, in_ap=e2, denom_ap=sm)
        else:
            rc = outp.tile([CH, 1], F32, tag="rc", name="rc")
            nc.vector.reciprocal(out=rc, in_=sm)
            nc.vector.tensor_scalar_mul(out=pr, in0=e2, scalar1=rc)
        nc.sync.dma_start(out=probs[b_, c * CH : (c + 1) * CH, :], in_=pr)

    # ---- prologue: batch 0/1 projections ----
    kq0 = emit_kq_proj(iot0)
    va0 = emit_v_proj(iot0)

    BS = {}  # batch states
    kq_next = {0: kq0}
    va_next = {0: va0}
    iots = {0: iot0}
    if iot1 is not None:
        iots[1] = iot1

    def new_batch_state(b):
        usa = work.tile([E, N], FP16, tag="usa", name="usa")
        usb = work.tile([E, N], FP16, tag="usb", name="usb")
        et = [
            etp.tile([CH, H * COLW], FP16, tag=f"et{mc}", name=f"et{mc}")
            for mc in range(NCH)
        ]
        return {
            "b": b,
            "usa": usa,
            "usb": usb,
            "et": et,
            "kq": kq_next[b],
            "va": va_next[b],
            "iot": iots[b],
            "paT": {},
            "pbT": {},
        }

    def emit_st(bs, mc, h0, h1):
        stt = stp.tile([128, COLW * 3], F32, tag="st", name="stt")
        kq = bs["kq"]
        for i, h in enumerate(range(h0, h1)):
            kX = kq["kTa"] if h < 4 else kq["kTb"]
            qX = kq["qTa"] if h < 4 else kq["qTb"]
            hl = h % 4
            nc.tensor.matmul(
                stt[0:CH, i * COLW : i * COLW + N],
                kX[32 * hl : 32 * hl + D, mc * CH : (mc + 1) * CH],
                qX[32 * hl : 32 * hl + D, :],
                start=True,
                stop=True,
                tile_position=(32 * hl, 0),
            )
        return stt

    def emit_mask_exp(bs, stt, mc, h0, h1):
        nh = h1 - h0
        if with_mask:
            for i in range(nh):
                nc.vector.tensor_add(
                    out=stt[0:CH, i * COLW : i * COLW + N],
                    in0=stt[0:CH, i * COLW : i * COLW + N],
                    in1=bs["iot"]["mT"][mc],
                )
        etv = bs["et"][mc].rearrange("p (i c) -> p i c", c=COLW)
        stv = stt[0:CH].rearrange("p (i c) -> p i c", c=COLW)
        nc.scalar.activation(
            out=etv[:, h0 : h0 + nh, 0:N],
            in_=stv[:, 0:nh, 0:N],
            func=AF.Exp,
        )

    def emit_av(bs, mc, h0, h1):
        paT, pbT = bs["paT"], bs["pbT"]
        va, et = bs["va"], bs["et"]
        usa, usb = bs["usa"], bs["usb"]
        if mc not in paT:
            paT[mc] = pap.tile([128, COLW], F32, tag="pa", name="paT")
            pbT[mc] = pbp.tile([128, COLW], F32, tag="pb", name="pbT")
        for h in range(h0, h1):
            pt = paT[mc] if h < 4 else pbT[mc]
            hl = h % 4
            nc.tensor.matmul(
                pt[hl * 32 : (hl + 1) * 32, 0:N],
                va[mc][:, h * 32 : (h + 1) * 32],
                et[mc][:, h * COLW : h * COLW + N],
                start=True,
                stop=True,
                tile_position=(0, 32 * hl),
            )
        with nc.allow_low_precision(reason="fp16 U accum"):
            if h1 == 4 or (h0 <= 3 < h1):  # bank-A heads complete for mc
                if mc == 0:
                    nc.vector.tensor_copy(out=usa, in_=paT[mc][:, 0:N])
                else:
                    nc.vector.tensor_add(out=usa, in0=usa, in1=paT[mc][:, 0:N])
            if h1 == H:  # bank-B heads complete for mc
                if mc == 0:
                    nc.vector.tensor_copy(out=usb, in_=pbT[mc][:, 0:N])
                else:
                    nc.vector.tensor_add(out=usb, in0=usb, in1=pbT[mc][:, 0:N])

    # ---- global wave pipeline: per iteration [exp(gi-1), ST(gi), AV(gi-2)],
    # hooks only at wjs where the pa/pb accumulation windows are closed ----
    wave_seq = [
        (b, mc, h0, h1)
        for b in range(bpc)
        for mc in range(NCH)
        for (h0, h1) in WAVES
    ]
    rec = {}  # global index -> (bs, stt, mc, h0, h1)
    for gi, (b, mc, h0, h1) in enumerate(wave_seq):
        wj = gi % WPB
        if wj == 0:
            BS[b] = new_batch_state(b)
        bs = BS[b]
        # exp of previous wave first (keeps stt-ring WAR ordering sound)
        if gi >= 1:
            pbs, pstt, pmc, ph0, ph1 = rec[gi - 1]
            emit_mask_exp(pbs, pstt, pmc, ph0, ph1)
        rec[gi] = (bs, emit_st(bs, mc, h0, h1), mc, h0, h1)
        if gi >= 2:
            abs_, _astt, amc, ah0, ah1 = rec.pop(gi - 2)
            emit_av(abs_, amc, ah0, ah1)
        # hooks (after exp/ST/AV of this iteration)
        if wj == 0 and b + 2 <= bpc - 1:
            iots[b + 2] = emit_dma_in(b + 2)
        elif wj == 1 and b >= 1:
            emit_norm(BS[b - 1])
        elif wj == 4 and b >= 1:
            emit_final_chunk(BS[b - 1], 0)
        elif wj == 7:
            if b >= 1:
                emit_final_chunk(BS[b - 1], 1)
            if b + 1 <= bpc - 1:
                kq_next[b + 1] = emit_kq_proj(iots[b + 1])
                va_next[b + 1] = emit_v_proj(iots[b + 1])
        elif wj == 10 and b >= 1:
            emit_final_chunk(BS[b - 1], 2)
            emit_final_chunk(BS[b - 1], 3)

    # tail flush
    gi = len(wave_seq)
    pbs, pstt, pmc, ph0, ph1 = rec[gi - 1]
    emit_mask_exp(pbs, pstt, pmc, ph0, ph1)
    for k in (gi - 2, gi - 1):
        abs_, _astt, amc, ah0, ah1 = rec.pop(k)
        emit_av(abs_, amc, ah0, ah1)
    emit_norm(BS[bpc - 1])
    for c in range(NCH):
        emit_final_chunk(BS[bpc - 1], c)


def build(bpc=BPC, with_mask=False, use_pool_nr=True):
    nc = bacc.Bacc("TRN2", target_bir_lowering=False, debug=False)
    shapes = {
        "xfT": (bpc, E, N),
        "q1T": (bpc, E, N),
        "q0T": (bpc, E, N),
        "Wq1A": (E, E),
        "Wq1B": (E, E),
        "Wq0A": (E, E),
        "Wq0B": (E, E),
        "WkA": (E, E),
        "WkB": (E, E),
        "Wv": (E, E),
        "WcA": (E, E),
        "WcB": (E, E),
        "MEXP": (E, E),
    }
    if with_mask:
        shapes["maskT"] = (bpc, N, N)
        shapes["maskN"] = (bpc, N, N)
    f32_names = {"maskT", "maskN"}
    ap = {
        k: nc.dram_tensor(
            k, list(s), F32 if k in f32_names else FP16, kind="ExternalInput"
        ).ap()
        for k, s in shapes.items()
    }
    probs = nc.dram_tensor("probs", [bpc, N, N], F32, kind="ExternalOutput").ap()
    with tile.TileContext(nc) as tc:
        with ExitStack() as ctx:
            _emit(ctx, tc, ap, probs, bpc, with_mask, use_pool_nr)
    nc.compile()
    return nc


def _pad_cols(W, half):
    """[E, 64] head-cols of `half` spread to [E, 128] at 32-col boundaries."""
    out = np.zeros((E, E), np.float16)
    for hl in range(4):
        h = half * 4 + hl
        out[:, 32 * hl : 32 * hl + D] = W[:, h * D : (h + 1) * D]
    return out


def host_constants(Wq1, Wq0, Wk, Wv, Wc, bc):
    Wq1 = np.asarray(Wq1, np.float32)
    Wq0 = np.asarray(Wq0, np.float32)
    Wks = np.asarray(Wk, np.float32) * 0.25
    Wc = np.asarray(Wc, np.float32)
    wca = np.zeros((E, E), np.float32)
    wcb = np.zeros((E, E), np.float32)
    for hl in range(4):
        wca[32 * hl : 32 * hl + D, :] = Wc[hl * D : (hl + 1) * D, :]
        wcb[32 * hl : 32 * hl + D, :] = Wc[(hl + 4) * D : (hl + 5) * D, :]
    # bias folded into the ota==1 row (band-0 rowsum row normalizes to 1)
    wca[16, :] = np.asarray(bc, np.float32)
    mexp = np.zeros((E, E), np.float16)
    for p in range(E):
        mexp[32 * (p // 32) + 16, p] = 1.0
    return {
        "Wq1A": _pad_cols(Wq1, 0),
        "Wq1B": _pad_cols(Wq1, 1),
        "Wq0A": _pad_cols(Wq0, 0),
        "Wq0B": _pad_cols(Wq0, 1),
        "WkA": _pad_cols(Wks, 0),
        "WkB": _pad_cols(Wks, 1),
        "Wv": np.asarray(Wv, np.float16),
        "WcA": wca.astype(np.float16),
        "WcB": wcb.astype(np.float16),
        "MEXP": mexp,
    }


def host_in_map(inputs, c, bpc=BPC, with_mask=False):
    """Per-core input dict for core c (batches c*bpc .. (c+1)*bpc)."""
    sl = slice(c * bpc, (c + 1) * bpc)
    x = np.asarray(inputs["encoded_nodes_f"], np.float32)[sl]
    q1 = np.asarray(inputs["encoded_q1_t"], np.float32)[sl]
    q0 = np.asarray(inputs["encoded_q0"], np.float32)[sl]
    m = {
        "xfT": np.ascontiguousarray(x.transpose(0, 2, 1)).astype(np.float16),
        "q1T": np.ascontiguousarray(q1.transpose(0, 2, 1)).astype(np.float16),
        "q0T": np.ascontiguousarray(q0.transpose(0, 2, 1)).astype(np.float16),
    }
    m.update(
        host_constants(
            inputs["Wq1"],
            inputs["Wq0"],
            inputs["Wk"],
            inputs["Wv"],
            inputs["Wc"],
            inputs["bc"],
        )
    )
    if with_mask:
        mask = np.asarray(inputs["ninf_mask"], np.float32)[sl]
        m["maskT"] = np.ascontiguousarray(mask.transpose(0, 2, 1))
        m["maskN"] = np.ascontiguousarray(mask)
    return m


_NC_CACHE = {}


def _get_nc(with_mask):
    if with_mask not in _NC_CACHE:
        _NC_CACHE[with_mask] = build(BPC, with_mask)
    return _NC_CACHE[with_mask]


def _ensure_ntff_hook():
    """Register the axon NTFF profile hook if the image's antenv lacks it."""
    import types

    try:
        from antenv.axon_hooks import get_axon_ntff_profile_hook  # noqa: F401

        return
    except ImportError:
        pass
    import antenv

    mod = types.ModuleType("antenv.axon_hooks")
    _h = {}
    mod.set_axon_ntff_profile_hook = lambda hook: _h.__setitem__("h", hook)
    mod.get_axon_ntff_profile_hook = lambda: _h.get("h")
    sys.modules["antenv.axon_hooks"] = mod
    antenv.axon_hooks = mod
    try:
        if "/root/.axon_site/trn_agent_boot" not in sys.path:
            sys.path.insert(0, "/root/.axon_site/trn_agent_boot")
        from trn_boot import _ntff_profile_via_ctypes

        mod.set_axon_ntff_profile_hook(
            _ntff_profile_via_ctypes("/opt/axon/libaxon_pjrt.so")
        )
    except Exception as e:  # degrade to no-trace
        print("ntff hook registration failed:", e)


def run(inputs, trace=False):
    """Run on 8 cores; returns (full probs array, BassKernelResults)."""
    from concourse.bass_utils import run_bass_kernel_spmd

    if trace:
        _ensure_ntff_hook()

    with_mask = bool(np.any(np.asarray(inputs["ninf_mask"])))
    nc = _get_nc(with_mask)
    in_maps = [host_in_map(inputs, c, BPC, with_mask) for c in range(NCORES)]
    res = run_bass_kernel_spmd(nc, in_maps, list(range(NCORES)), trace=trace)
    out = np.empty((B, N, N), np.float32)
    for c in range(NCORES):
        out[c * BPC : (c + 1) * BPC] = res.results[c]["probs"]
    return out, res


def kernel(**inputs):
    out, _ = run(inputs)
    return out


# revision 3
# speedup vs baseline: 1.0016x; 1.0016x over previous
"""Trainium2 Bass kernel for nn_Decoder_45380624450003.

Multi-head attention decoder + single-head pointer attention, data-parallel
over the batch dim across 8 NeuronCores (8 batches per core).

v3: global wave pipeline across all batches. Per global wave w the emission
order is [exp(w-1), ST(w), AV(w-2)] so the ACT engine never waits on a
just-signalled semaphore and the PE stream stays dense across batch
boundaries. k/q/v projection PSUM lives in the pa/pb single-bank "highway"
(the 6-bank ST ring stays pure so its 2-buffer parity holds). Weight DMAs
ride the idle GpSimd queue; input DMAs keep the SP queue. Final softmax
normalize runs on GpSimd (normalize_recip). Engine warmup releases the
PE/ACT activity throttles during the initial DMA wait.
"""

import sys

sys.path.insert(0, "/opt/trn_rl_repo")

from contextlib import ExitStack

import numpy as np

import concourse.bacc as bacc
import concourse.tile as tile
from concourse import mybir

F32 = mybir.dt.float32
FP16 = mybir.dt.float16
AF = mybir.ActivationFunctionType

B, N, E, H, D = 64, 500, 128, 8, 16
NCORES = 8
BPC = B // NCORES  # batches per core
NCH = 4
CH = N // NCH  # 125 rows per n/m chunk
SQRT_EMB = 11.313708498984761
CLIP = 10.0
COLW = 512  # psum bank width in f32
WAVES = [(0, 3), (3, 6), (6, 8)]  # head ranges per ST/exp wave
WPB = len(WAVES) * NCH  # waves per batch (12)


def _emit(ctx, tc, ap, probs, bpc, with_mask, use_pool_nr=True):
    nc = tc.nc

    const = ctx.enter_context(tc.tile_pool(name="const", bufs=1))
    io = ctx.enter_context(tc.tile_pool(name="io", bufs=4))
    proj = ctx.enter_context(tc.tile_pool(name="proj", bufs=2))
    etp = ctx.enter_context(tc.tile_pool(name="et", bufs=1))
    work = ctx.enter_context(tc.tile_pool(name="work", bufs=2))
    outp = ctx.enter_context(tc.tile_pool(name="outp", bufs=3))
    stp = ctx.enter_context(tc.tile_pool(name="st", bufs=2, space="PSUM"))
    pap = ctx.enter_context(tc.tile_pool(name="pa", bufs=1, space="PSUM"))
    pbp = ctx.enter_context(tc.tile_pool(name="pb", bufs=1, space="PSUM"))

    # ---- input DMAs for batch 0 first (SP queue), weights on GpSimd queue --
    def emit_dma_in(b):
        xf = io.tile([E, N], FP16, tag="xf", name="xf")
        q1 = io.tile([E, N], FP16, tag="q1", name="q1")
        q0 = io.tile([E, N], FP16, tag="q0", name="q0")
        nc.sync.dma_start(out=xf, in_=ap["xfT"][b])
        nc.sync.dma_start(out=q1, in_=ap["q1T"][b])
        nc.sync.dma_start(out=q0, in_=ap["q0T"][b])
        mT = mN = None
        if with_mask:
            mT = [io.tile([CH, N], F32, tag=f"mT{i}", name=f"mT{i}") for i in range(NCH)]
            mN = [io.tile([CH, N], F32, tag=f"mN{i}", name=f"mN{i}") for i in range(NCH)]
            for mc in range(NCH):
                nc.sync.dma_start(
                    out=mT[mc], in_=ap["maskT"][b, mc * CH : (mc + 1) * CH, :]
                )
                nc.sync.dma_start(
                    out=mN[mc], in_=ap["maskN"][b, mc * CH : (mc + 1) * CH, :]
                )
        return {"xf": xf, "q1": q1, "q0": q0, "mT": mT, "mN": mN}

    iot0 = emit_dma_in(0)

    w = {}
    for k in ["WkA", "WkB", "Wq1A", "Wq1B", "Wq0A", "Wq0B", "Wv", "WcA", "WcB",
              "MEXP"]:
        w[k] = const.tile([E, E], FP16, tag=k, name=k)
        nc.gpsimd.dma_start(out=w[k], in_=ap[k])

    iot1 = emit_dma_in(1) if bpc > 1 else None

    # ---- engine warmup: release PE/ACT activity throttles during DMA wait --
    warm = const.tile([E, COLW], FP16, tag="warm", name="warm")
    nc.vector.memset(warm[:, :], 0.125)
    wsb = const.tile([E, 2048], FP16, tag="wsb", name="wsb")
    nc.vector.memset(wsb[:, :], 0.25)
    for i in range(10):
        pool = pap if i % 2 == 0 else pbp
        tag = "pa" if i % 2 == 0 else "pb"
        wp = pool.tile([128, COLW], F32, tag=tag, name="warmps")
        nc.tensor.matmul(
            wp[:, 0:N], warm[:, 0:128], warm[:, 0:N], start=True, stop=True
        )
    nc.scalar.activation(out=wsb[:, 0:500], in_=wsb[:, 0:500], func=AF.Tanh)
    for i in range(2):
        nc.scalar.activation(out=wsb[:, 0:2000], in_=wsb[:, 0:2000], func=AF.Exp)

    def emit_kq_proj(iot):
        """k/q projections in the pa/pb highway + casts to fp16 SBUF."""
        xf, q1, q0 = iot["xf"], iot["q1"], iot["q0"]
        kpsA = pap.tile([128, COLW], F32, tag="pa", name="kpsA")
        kpsB = pbp.tile([128, COLW], F32, tag="pb", name="kpsB")
        nc.tensor.matmul(kpsA[:, 0:N], w["WkA"], xf, start=True, stop=True)
        nc.tensor.matmul(kpsB[:, 0:N], w["WkB"], xf, start=True, stop=True)
        kTa = proj.tile([E, N], FP16, tag="kTa", name="kTa")
        kTb = proj.tile([E, N], FP16, tag="kTb", name="kTb")
        nc.vector.tensor_copy(out=kTa, in_=kpsA[:, 0:N])
        nc.vector.tensor_copy(out=kTb, in_=kpsB[:, 0:N])

        qpsA = pap.tile([128, COLW], F32, tag="pa", name="qpsA")
        qpsB = pbp.tile([128, COLW], F32, tag="pb", name="qpsB")
        nc.tensor.matmul(qpsA[:, 0:N], w["Wq1A"], q1, start=True, stop=False)
        nc.tensor.matmul(qpsA[:, 0:N], w["Wq0A"], q0, start=False, stop=True)
        nc.tensor.matmul(qpsB[:, 0:N], w["Wq1B"], q1, start=True, stop=False)
        nc.tensor.matmul(qpsB[:, 0:N], w["Wq0B"], q0, start=False, stop=True)
        qTa = proj.tile([E, N], FP16, tag="qTa", name="qTa")
        qTb = proj.tile([E, N], FP16, tag="qTb", name="qTb")
        nc.vector.tensor_copy(out=qTa, in_=qpsA[:, 0:N])
        nc.vector.tensor_copy(out=qTb, in_=qpsB[:, 0:N])
        return {"kTa": kTa, "kTb": kTb, "qTa": qTa, "qTb": qTb}

    def emit_v_proj(iot):
        """v projection: vps fits one PSUM bank (4 chunks x 128 cols)."""
        xf = iot["xf"]
        vps = pap.tile([128, COLW], F32, tag="pa", name="vps")
        for mc in range(NCH):
            nc.tensor.matmul(
                vps[0:CH, mc * E : (mc + 1) * E],
                xf[:, mc * CH : (mc + 1) * CH],
                w["Wv"],
                start=True,
                stop=True,
            )
        va = [
            proj.tile([CH, H * 32], FP16, tag=f"va{mc}", name=f"va{mc}")
            for mc in range(NCH)
        ]
        for mc in range(NCH):
            var = va[mc].rearrange("p (h c) -> p h c", h=H)
            nc.vector.memset(va[mc][:, :], 0.0)
            nc.vector.memset(var[:, :, 16:17], 1.0)
            nc.vector.tensor_copy(
                out=var[:, :, 0:16],
                in_=vps[0:CH, mc * E : (mc + 1) * E].rearrange(
                    "p (h d) -> p h d", h=H
                ),
            )
        return va

    def emit_norm(bs):
        """Merged normalization: expander matmul -> reciprocal -> normalize
        -> combine (bias folded into WcA row 16 via the ones row of ota)."""
        usa, usb = bs["usa"], bs["usb"]
        rsA = pap.tile([128, COLW], F32, tag="pa", name="rsA")
        rsB = pbp.tile([128, COLW], F32, tag="pb", name="rsB")
        nc.tensor.matmul(rsA[:, 0:N], w["MEXP"], usa, start=True, stop=True)
        nc.tensor.matmul(rsB[:, 0:N], w["MEXP"], usb, start=True, stop=True)
        recA = work.tile([E, N], F32, tag="recA", name="recA")
        recB = work.tile([E, N], F32, tag="recB", name="recB")
        with nc.allow_low_precision(reason="fp16 softmax normalize"):
            nc.vector.reciprocal_approx_fast(out=recA, in_=rsA[:, 0:N])
            nc.vector.reciprocal_approx_fast(out=recB, in_=rsB[:, 0:N])
            ota = work.tile([E, N], FP16, tag="ota", name="ota")
            otb = work.tile([E, N], FP16, tag="otb", name="otb")
            nc.vector.tensor_mul(out=ota, in0=usa, in1=recA)
            nc.vector.tensor_mul(out=otb, in0=usb, in1=recB)
        mps = pap.tile([128, COLW], F32, tag="pa", name="mps")
        nc.tensor.matmul(mps[:, 0:N], w["WcA"], ota, start=True, stop=False)
        nc.tensor.matmul(mps[:, 0:N], w["WcB"], otb, start=False, stop=True)
        mh = work.tile([E, N], FP16, tag="mh", name="mh")
        with nc.allow_low_precision(reason="fp16 mh for pointer matmul"):
            nc.vector.tensor_copy(out=mh, in_=mps[:, 0:N])
        bs["mh"] = mh

    def emit_final_chunk(bs, c):
        """Pointer attention for chunk c of a finished batch."""
        mh, xf_, mN_ = bs["mh"], bs["iot"]["xf"], bs["iot"]["mN"]
        b_ = bs["b"]
        pool = pap if c % 2 == 0 else pbp
        tag = "pa" if c % 2 == 0 else "pb"
        sps = pool.tile([128, COLW], F32, tag=tag, name="sps")
        nc.tensor.matmul(
            sps[0:CH, 0:N],
            mh[:, c * CH : (c + 1) * CH],
            xf_,
            start=True,
            stop=True,
        )
        th = outp.tile([CH, N], F32, tag="th", name="th")
        nc.scalar.activation(
            out=th, in_=sps[0:CH, 0:N], func=AF.Tanh, scale=1.0 / SQRT_EMB
        )
        e2 = outp.tile([CH, N], F32, tag="e2", name="e2")
        sm = outp.tile([CH, 1], F32, tag="sm", name="sm")
        if with_mask:
            tm = outp.tile([CH, N], F32, tag="tm", name="tm")
            nc.vector.scalar_tensor_tensor(
                out=tm,
                in0=th,
                scalar=CLIP,
                in1=mN_[c],
                op0=mybir.AluOpType.mult,
                op1=mybir.AluOpType.add,
            )
            nc.scalar.activation(out=e2, in_=tm, func=AF.Exp, accum_out=sm)
        else:
            nc.scalar.activation(
                out=e2, in_=th, func=AF.Exp, scale=CLIP, accum_out=sm
            )
        pr = outp.tile([CH, N], FP16, tag="pr", name="pr")
        with nc.allow_low_precision(reason="fp16 probs; host upcasts"):
            if use_pool_nr:
                nc.gpsimd.normalize_recip(out_ap=pr, in_ap=e2, denom_ap=sm)
            else:
                rc = outp.tile([CH, 1], F32, tag="rc", name="rc")
                nc.vector.reciprocal(out=rc, in_=sm)
                nc.vector.tensor_scalar_mul(out=pr, in0=e2, scalar1=rc)
        nc.sync.dma_start(out=probs[b_, c * CH : (c + 1) * CH, :], in_=pr)

    # ---- prologue: batch 0/1 projections ----
    kq0 = emit_kq_proj(iot0)
    va0 = emit_v_proj(iot0)

    BS = {}  # batch states
    kq_next = {0: kq0}
    va_next = {0: va0}
    iots = {0: iot0}
    if iot1 is not None:
        iots[1] = iot1

    def new_batch_state(b):
        usa = work.tile([E, N], FP16, tag="usa", name="usa")
        usb = work.tile([E, N], FP16, tag="usb", name="usb")
        et = [
            etp.tile([CH, H * COLW], FP16, tag=f"et{mc}", name=f"et{mc}")
            for mc in range(NCH)
        ]
        return {
            "b": b,
            "usa": usa,
            "usb": usb,
            "et": et,
            "kq": kq_next[b],
            "va": va_next[b],
            "iot": iots[b],
            "paT": {},
            "pbT": {},
        }

    def emit_st(bs, mc, h0, h1):
        stt = stp.tile([128, COLW * 3], F32, tag="st", name="stt")
        kq = bs["kq"]
        for i, h in enumerate(range(h0, h1)):
            kX = kq["kTa"] if h < 4 else kq["kTb"]
            qX = kq["qTa"] if h < 4 else kq["qTb"]
            hl = h % 4
            nc.tensor.matmul(
                stt[0:CH, i * COLW : i * COLW + N],
                kX[32 * hl : 32 * hl + D, mc * CH : (mc + 1) * CH],
                qX[32 * hl : 32 * hl + D, :],
                start=True,
                stop=True,
                tile_position=(32 * hl, 0),
            )
        return stt

    def emit_mask_exp(bs, stt, mc, h0, h1):
        nh = h1 - h0
        if with_mask:
            for i in range(nh):
                nc.vector.tensor_add(
                    out=stt[0:CH, i * COLW : i * COLW + N],
                    in0=stt[0:CH, i * COLW : i * COLW + N],
                    in1=bs["iot"]["mT"][mc],
                )
        etv = bs["et"][mc].rearrange("p (i c) -> p i c", c=COLW)
        stv = stt[0:CH].rearrange("p (i c) -> p i c", c=COLW)
        nc.scalar.activation(
            out=etv[:, h0 : h0 + nh, 0:N],
            in_=stv[:, 0:nh, 0:N],
            func=AF.Exp,
        )

    def emit_av(bs, mc, h0, h1):
        paT, pbT = bs["paT"], bs["pbT"]
        va, et = bs["va"], bs["et"]
        usa, usb = bs["usa"], bs["usb"]
        if mc not in paT:
            paT[mc] = pap.tile([128, COLW], F32, tag="pa", name="paT")
            pbT[mc] = pbp.tile([128, COLW], F32, tag="pb", name="pbT")
        for h in range(h0, h1):
            pt = paT[mc] if h < 4 else pbT[mc]
            hl = h % 4
            nc.tensor.matmul(
                pt[hl * 32 : (hl + 1) * 32, 0:N],
                va[mc][:, h * 32 : (h + 1) * 32],
                et[mc][:, h * COLW : h * COLW + N],
                start=True,
                stop=True,
                tile_position=(0, 32 * hl),
            )
        with nc.allow_low_precision(reason="fp16 U accum"):
            if h1 == 4 or (h0 <= 3 < h1):  # bank-A heads complete for mc
                if mc == 0:
                    nc.vector.tensor_copy(out=usa, in_=paT[mc][:, 0:N])
                else:
                    nc.vector.tensor_add(out=usa, in0=usa, in1=paT[mc][:, 0:N])
            if h1 == H:  # bank-B heads complete for mc
                if mc == 0:
                    nc.vector.tensor_copy(out=usb, in_=pbT[mc][:, 0:N])
                else:
                    nc.vector.tensor_add(out=usb, in0=usb, in1=pbT[mc][:, 0:N])

    # ---- global wave pipeline: per iteration [exp(gi-1), ST(gi), AV(gi-2)],
    # hooks only at wjs where the pa/pb accumulation windows are closed ----
    wave_seq = [
        (b, mc, h0, h1)
        for b in range(bpc)
        for mc in range(NCH)
        for (h0, h1) in WAVES
    ]
    rec = {}  # global index -> (bs, stt, mc, h0, h1)
    for gi, (b, mc, h0, h1) in enumerate(wave_seq):
        wj = gi % WPB
        if wj == 0:
            BS[b] = new_batch_state(b)
        bs = BS[b]
        # exp of previous wave first (keeps stt-ring WAR ordering sound)
        if gi >= 1:
            pbs, pstt, pmc, ph0, ph1 = rec[gi - 1]
            emit_mask_exp(pbs, pstt, pmc, ph0, ph1)
        rec[gi] = (bs, emit_st(bs, mc, h0, h1), mc, h0, h1)
        if gi >= 2:
            abs_, _astt, amc, ah0, ah1 = rec.pop(gi - 2)
            emit_av(abs_, amc, ah0, ah1)
        # hooks (after exp/ST/AV of this iteration)
        if wj == 0 and b + 2 <= bpc - 1:
            iots[b + 2] = emit_dma_in(b + 2)
        elif wj == 1 and b >= 1:
            emit_norm(BS[b - 1])
        elif wj == 4 and b >= 1:
            emit_final_chunk(BS[b - 1], 0)
        elif wj == 7:
            if b >= 1:
                emit_final_chunk(BS[b - 1], 1)
            if b + 1 <= bpc - 1:
                kq_next[b + 1] = emit_kq_proj(iots[b + 1])
                va_next[b + 1] = emit_v_proj(iots[b + 1])
        elif wj == 10 and b >= 1:
            emit_final_chunk(BS[b - 1], 2)
            emit_final_chunk(BS[b - 1], 3)

    # tail flush
    gi = len(wave_seq)
    pbs, pstt, pmc, ph0, ph1 = rec[gi - 1]
    emit_mask_exp(pbs, pstt, pmc, ph0, ph1)
    for k in (gi - 2, gi - 1):
        abs_, _astt, amc, ah0, ah1 = rec.pop(k)
        emit_av(abs_, amc, ah0, ah1)
    emit_norm(BS[bpc - 1])
    for c in range(NCH):
        emit_final_chunk(BS[bpc - 1], c)


def build(bpc=BPC, with_mask=False, use_pool_nr=True):
    nc = bacc.Bacc("TRN2", target_bir_lowering=False, debug=False)
    shapes = {
        "xfT": (bpc, E, N),
        "q1T": (bpc, E, N),
        "q0T": (bpc, E, N),
        "Wq1A": (E, E),
        "Wq1B": (E, E),
        "Wq0A": (E, E),
        "Wq0B": (E, E),
        "WkA": (E, E),
        "WkB": (E, E),
        "Wv": (E, E),
        "WcA": (E, E),
        "WcB": (E, E),
        "MEXP": (E, E),
    }
    if with_mask:
        shapes["maskT"] = (bpc, N, N)
        shapes["maskN"] = (bpc, N, N)
    f32_names = {"maskT", "maskN"}
    ap = {
        k: nc.dram_tensor(
            k, list(s), F32 if k in f32_names else FP16, kind="ExternalInput"
        ).ap()
        for k, s in shapes.items()
    }
    probs = nc.dram_tensor("probs", [bpc, N, N], FP16, kind="ExternalOutput").ap()
    with tile.TileContext(nc) as tc:
        with ExitStack() as ctx:
            _emit(ctx, tc, ap, probs, bpc, with_mask, use_pool_nr)
    nc.compile()
    return nc


def _pad_cols(W, half):
    """[E, 64] head-cols of `half` spread to [E, 128] at 32-col boundaries."""
    out = np.zeros((E, E), np.float16)
    for hl in range(4):
        h = half * 4 + hl
        out[:, 32 * hl : 32 * hl + D] = W[:, h * D : (h + 1) * D]
    return out


def host_constants(Wq1, Wq0, Wk, Wv, Wc, bc):
    Wq1 = np.asarray(Wq1, np.float32)
    Wq0 = np.asarray(Wq0, np.float32)
    Wks = np.asarray(Wk, np.float32) * 0.25
    Wc = np.asarray(Wc, np.float32)
    wca = np.zeros((E, E), np.float32)
    wcb = np.zeros((E, E), np.float32)
    for hl in range(4):
        wca[32 * hl : 32 * hl + D, :] = Wc[hl * D : (hl + 1) * D, :]
        wcb[32 * hl : 32 * hl + D, :] = Wc[(hl + 4) * D : (hl + 5) * D, :]
    # bias folded into the ota==1 row (band-0 rowsum row normalizes to 1)
    wca[16, :] = np.asarray(bc, np.float32)
    mexp = np.zeros((E, E), np.float16)
    for p in range(E):
        mexp[32 * (p // 32) + 16, p] = 1.0
    return {
        "Wq1A": _pad_cols(Wq1, 0),
        "Wq1B": _pad_cols(Wq1, 1),
        "Wq0A": _pad_cols(Wq0, 0),
        "Wq0B": _pad_cols(Wq0, 1),
        "WkA": _pad_cols(Wks, 0),
        "WkB": _pad_cols(Wks, 1),
        "Wv": np.asarray(Wv, np.float16),
        "WcA": wca.astype(np.float16),
        "WcB": wcb.astype(np.float16),
        "MEXP": mexp,
    }


def host_in_map(inputs, c, bpc=BPC, with_mask=False):
    """Per-core input dict for core c (batches c*bpc .. (c+1)*bpc)."""
    sl = slice(c * bpc, (c + 1) * bpc)
    x = np.asarray(inputs["encoded_nodes_f"], np.float32)[sl]
    q1 = np.asarray(inputs["encoded_q1_t"], np.float32)[sl]
    q0 = np.asarray(inputs["encoded_q0"], np.float32)[sl]
    m = {
        "xfT": np.ascontiguousarray(x.transpose(0, 2, 1)).astype(np.float16),
        "q1T": np.ascontiguousarray(q1.transpose(0, 2, 1)).astype(np.float16),
        "q0T": np.ascontiguousarray(q0.transpose(0, 2, 1)).astype(np.float16),
    }
    m.update(
        host_constants(
            inputs["Wq1"],
            inputs["Wq0"],
            inputs["Wk"],
            inputs["Wv"],
            inputs["Wc"],
            inputs["bc"],
        )
    )
    if with_mask:
        mask = np.asarray(inputs["ninf_mask"], np.float32)[sl]
        m["maskT"] = np.ascontiguousarray(mask.transpose(0, 2, 1))
        m["maskN"] = np.ascontiguousarray(mask)
    return m


_NC_CACHE = {}


def _get_nc(with_mask):
    if with_mask not in _NC_CACHE:
        _NC_CACHE[with_mask] = build(BPC, with_mask)
    return _NC_CACHE[with_mask]


def _ensure_ntff_hook():
    """Register the axon NTFF profile hook if the image's antenv lacks it."""
    import types

    try:
        from antenv.axon_hooks import get_axon_ntff_profile_hook  # noqa: F401

        return
    except ImportError:
        pass
    import antenv

    mod = types.ModuleType("antenv.axon_hooks")
    _h = {}
    mod.set_axon_ntff_profile_hook = lambda hook: _h.__setitem__("h", hook)
    mod.get_axon_ntff_profile_hook = lambda: _h.get("h")
    sys.modules["antenv.axon_hooks"] = mod
    antenv.axon_hooks = mod
    try:
        if "/root/.axon_site/trn_agent_boot" not in sys.path:
            sys.path.insert(0, "/root/.axon_site/trn_agent_boot")
        from trn_boot import _ntff_profile_via_ctypes

        mod.set_axon_ntff_profile_hook(
            _ntff_profile_via_ctypes("/opt/axon/libaxon_pjrt.so")
        )
    except Exception as e:  # degrade to no-trace
        print("ntff hook registration failed:", e)


def run(inputs, trace=False):
    """Run on 8 cores; returns (full probs array, BassKernelResults)."""
    from concourse.bass_utils import run_bass_kernel_spmd

    if trace:
        _ensure_ntff_hook()

    with_mask = bool(np.any(np.asarray(inputs["ninf_mask"])))
    nc = _get_nc(with_mask)
    in_maps = [host_in_map(inputs, c, BPC, with_mask) for c in range(NCORES)]
    res = run_bass_kernel_spmd(nc, in_maps, list(range(NCORES)), trace=trace)
    out = np.empty((B, N, N), np.float32)
    for c in range(NCORES):
        out[c * BPC : (c + 1) * BPC] = res.results[c]["probs"].astype(np.float32)
    return out, res


def kernel(**inputs):
    out, _ = run(inputs)
    return out


# revision 4
# speedup vs baseline: 1.0016x; 1.0001x over previous
"""Trainium2 Bass kernel for nn_Decoder_45380624450003.

Multi-head attention decoder + single-head pointer attention, data-parallel
over the batch dim across 8 NeuronCores (8 batches per core).

v3: global wave pipeline across all batches. Per global wave w the emission
order is [exp(w-1), ST(w), AV(w-2)] so the ACT engine never waits on a
just-signalled semaphore and the PE stream stays dense across batch
boundaries. k/q/v projection PSUM lives in the pa/pb single-bank "highway"
(the 6-bank ST ring stays pure so its 2-buffer parity holds). Weight DMAs
ride the idle GpSimd queue; input DMAs keep the SP queue. Final softmax
normalize runs on GpSimd (normalize_recip). Engine warmup releases the
PE/ACT activity throttles during the initial DMA wait.
"""

import sys

sys.path.insert(0, "/opt/trn_rl_repo")

from contextlib import ExitStack

import numpy as np

import concourse.bacc as bacc
import concourse.tile as tile
from concourse import mybir

F32 = mybir.dt.float32
FP16 = mybir.dt.float16
AF = mybir.ActivationFunctionType

B, N, E, H, D = 64, 500, 128, 8, 16
NCORES = 8
BPC = B // NCORES  # batches per core
NCH = 4
CH = N // NCH  # 125 rows per n/m chunk
SQRT_EMB = 11.313708498984761
CLIP = 10.0
COLW = 512  # psum bank width in f32
WAVES = [(0, 3), (3, 6), (6, 8)]  # head ranges per ST/exp wave
WPB = len(WAVES) * NCH  # waves per batch (12)


def _emit(ctx, tc, ap, probs, bpc, with_mask, use_pool_nr=True):
    nc = tc.nc

    const = ctx.enter_context(tc.tile_pool(name="const", bufs=1))
    io = ctx.enter_context(tc.tile_pool(name="io", bufs=4))
    proj = ctx.enter_context(tc.tile_pool(name="proj", bufs=2))
    etp = ctx.enter_context(tc.tile_pool(name="et", bufs=1))
    work = ctx.enter_context(tc.tile_pool(name="work", bufs=2))
    outp = ctx.enter_context(tc.tile_pool(name="outp", bufs=3))
    stp = ctx.enter_context(tc.tile_pool(name="st", bufs=2, space="PSUM"))
    pap = ctx.enter_context(tc.tile_pool(name="pa", bufs=1, space="PSUM"))
    pbp = ctx.enter_context(tc.tile_pool(name="pb", bufs=1, space="PSUM"))

    # ---- input DMAs for batch 0 first (SP queue), weights on GpSimd queue --
    def emit_dma_in(b):
        xf = io.tile([E, N], FP16, tag="xf", name="xf")
        q1 = io.tile([E, N], FP16, tag="q1", name="q1")
        q0 = io.tile([E, N], FP16, tag="q0", name="q0")
        nc.sync.dma_start(out=xf, in_=ap["xfT"][b])
        nc.sync.dma_start(out=q1, in_=ap["q1T"][b])
        nc.sync.dma_start(out=q0, in_=ap["q0T"][b])
        mT = mN = None
        if with_mask:
            mT = [io.tile([CH, N], F32, tag=f"mT{i}", name=f"mT{i}") for i in range(NCH)]
            mN = [io.tile([CH, N], F32, tag=f"mN{i}", name=f"mN{i}") for i in range(NCH)]
            for mc in range(NCH):
                nc.sync.dma_start(
                    out=mT[mc], in_=ap["maskT"][b, mc * CH : (mc + 1) * CH, :]
                )
                nc.sync.dma_start(
                    out=mN[mc], in_=ap["maskN"][b, mc * CH : (mc + 1) * CH, :]
                )
        return {"xf": xf, "q1": q1, "q0": q0, "mT": mT, "mN": mN}

    iot0 = emit_dma_in(0)

    w = {}
    for k in ["WkA", "WkB", "Wq1A", "Wq1B", "Wq0A", "Wq0B", "Wv", "WcA", "WcB",
              "MEXP"]:
        w[k] = const.tile([E, E], FP16, tag=k, name=k)
        nc.gpsimd.dma_start(out=w[k], in_=ap[k])

    iot1 = emit_dma_in(1) if bpc > 1 else None

    # ---- engine warmup: release PE/ACT activity throttles during DMA wait --
    warm = const.tile([E, COLW], FP16, tag="warm", name="warm")
    nc.vector.memset(warm[:, :], 0.125)
    wsb = const.tile([E, 2048], FP16, tag="wsb", name="wsb")
    nc.vector.memset(wsb[:, :], 0.25)
    for i in range(10):
        pool = pap if i % 2 == 0 else pbp
        tag = "pa" if i % 2 == 0 else "pb"
        wp = pool.tile([128, COLW], F32, tag=tag, name="warmps")
        nc.tensor.matmul(
            wp[:, 0:N], warm[:, 0:128], warm[:, 0:N], start=True, stop=True
        )
    nc.scalar.activation(out=wsb[:, 0:500], in_=wsb[:, 0:500], func=AF.Tanh)
    for i in range(2):
        nc.scalar.activation(out=wsb[:, 0:2000], in_=wsb[:, 0:2000], func=AF.Exp)

    def emit_kq_proj(iot):
        """k/q projections in the pa/pb highway + casts to fp16 SBUF."""
        xf, q1, q0 = iot["xf"], iot["q1"], iot["q0"]
        kpsA = pap.tile([128, COLW], F32, tag="pa", name="kpsA")
        kpsB = pbp.tile([128, COLW], F32, tag="pb", name="kpsB")
        nc.tensor.matmul(kpsA[:, 0:N], w["WkA"], xf, start=True, stop=True)
        nc.tensor.matmul(kpsB[:, 0:N], w["WkB"], xf, start=True, stop=True)
        kTa = proj.tile([E, N], FP16, tag="kTa", name="kTa")
        kTb = proj.tile([E, N], FP16, tag="kTb", name="kTb")
        nc.vector.tensor_copy(out=kTa, in_=kpsA[:, 0:N])
        nc.vector.tensor_copy(out=kTb, in_=kpsB[:, 0:N])

        qpsA = pap.tile([128, COLW], F32, tag="pa", name="qpsA")
        qpsB = pbp.tile([128, COLW], F32, tag="pb", name="qpsB")
        nc.tensor.matmul(qpsA[:, 0:N], w["Wq1A"], q1, start=True, stop=False)
        nc.tensor.matmul(qpsA[:, 0:N], w["Wq0A"], q0, start=False, stop=True)
        nc.tensor.matmul(qpsB[:, 0:N], w["Wq1B"], q1, start=True, stop=False)
        nc.tensor.matmul(qpsB[:, 0:N], w["Wq0B"], q0, start=False, stop=True)
        qTa = proj.tile([E, N], FP16, tag="qTa", name="qTa")
        qTb = proj.tile([E, N], FP16, tag="qTb", name="qTb")
        nc.vector.tensor_copy(out=qTa, in_=qpsA[:, 0:N])
        nc.vector.tensor_copy(out=qTb, in_=qpsB[:, 0:N])
        return {"kTa": kTa, "kTb": kTb, "qTa": qTa, "qTb": qTb}

    def emit_v_proj(iot):
        """v projection: vps fits one PSUM bank (4 chunks x 128 cols)."""
        xf = iot["xf"]
        vps = pap.tile([128, COLW], F32, tag="pa", name="vps")
        for mc in range(NCH):
            nc.tensor.matmul(
                vps[0:CH, mc * E : (mc + 1) * E],
                xf[:, mc * CH : (mc + 1) * CH],
                w["Wv"],
                start=True,
                stop=True,
            )
        va = [
            proj.tile([CH, H * 32], FP16, tag=f"va{mc}", name=f"va{mc}")
            for mc in range(NCH)
        ]
        for mc in range(NCH):
            var = va[mc].rearrange("p (h c) -> p h c", h=H)
            nc.vector.memset(va[mc][:, :], 0.0)
            nc.vector.memset(var[:, :, 16:17], 1.0)
            nc.vector.tensor_copy(
                out=var[:, :, 0:16],
                in_=vps[0:CH, mc * E : (mc + 1) * E].rearrange(
                    "p (h d) -> p h d", h=H
                ),
            )
        return va

    def emit_norm(bs):
        """Merged normalization: expander matmul -> reciprocal -> normalize
        -> combine (bias folded into WcA row 16 via the ones row of ota)."""
        usa, usb = bs["usa"], bs["usb"]
        rsA = pap.tile([128, COLW], F32, tag="pa", name="rsA")
        rsB = pbp.tile([128, COLW], F32, tag="pb", name="rsB")
        nc.tensor.matmul(rsA[:, 0:N], w["MEXP"], usa, start=True, stop=True)
        nc.tensor.matmul(rsB[:, 0:N], w["MEXP"], usb, start=True, stop=True)
        recA = work.tile([E, N], F32, tag="recA", name="recA")
        recB = work.tile([E, N], F32, tag="recB", name="recB")
        with nc.allow_low_precision(reason="fp16 softmax normalize"):
            nc.vector.reciprocal_approx_fast(out=recA, in_=rsA[:, 0:N])
            nc.vector.reciprocal_approx_fast(out=recB, in_=rsB[:, 0:N])
            ota = work.tile([E, N], FP16, tag="ota", name="ota")
            otb = work.tile([E, N], FP16, tag="otb", name="otb")
            nc.vector.tensor_mul(out=ota, in0=usa, in1=recA)
            nc.vector.tensor_mul(out=otb, in0=usb, in1=recB)
        mps = pap.tile([128, COLW], F32, tag="pa", name="mps")
        nc.tensor.matmul(mps[:, 0:N], w["WcA"], ota, start=True, stop=False)
        nc.tensor.matmul(mps[:, 0:N], w["WcB"], otb, start=False, stop=True)
        mh = work.tile([E, N], FP16, tag="mh", name="mh")
        with nc.allow_low_precision(reason="fp16 mh for pointer matmul"):
            nc.vector.tensor_copy(out=mh, in_=mps[:, 0:N])
        bs["mh"] = mh

    def emit_final_chunk(bs, c, mh_c=None):
        """Pointer attention for chunk c of a finished batch."""
        mh, xf_, mN_ = bs["mh"], bs["iot"]["xf"], bs["iot"]["mN"]
        b_ = bs["b"]
        if mh_c is None:
            mh_c = c
        pool = pap if c % 2 == 0 else pbp
        tag = "pa" if c % 2 == 0 else "pb"
        sps = pool.tile([128, COLW], F32, tag=tag, name="sps")
        nc.tensor.matmul(
            sps[0:CH, 0:N],
            mh[:, mh_c * CH : (mh_c + 1) * CH],
            xf_,
            start=True,
            stop=True,
        )
        th = outp.tile([CH, N], F32, tag="th", name="th")
        nc.scalar.activation(
            out=th, in_=sps[0:CH, 0:N], func=AF.Tanh, scale=1.0 / SQRT_EMB
        )
        e2 = outp.tile([CH, N], F32, tag="e2", name="e2")
        sm = outp.tile([CH, 1], F32, tag="sm", name="sm")
        if with_mask:
            tm = outp.tile([CH, N], F32, tag="tm", name="tm")
            nc.vector.scalar_tensor_tensor(
                out=tm,
                in0=th,
                scalar=CLIP,
                in1=mN_[c],
                op0=mybir.AluOpType.mult,
                op1=mybir.AluOpType.add,
            )
            nc.scalar.activation(out=e2, in_=tm, func=AF.Exp, accum_out=sm)
        else:
            nc.scalar.activation(
                out=e2, in_=th, func=AF.Exp, scale=CLIP, accum_out=sm
            )
        pr = outp.tile([CH, N], FP16, tag="pr", name="pr")
        with nc.allow_low_precision(reason="fp16 probs; host upcasts"):
            if use_pool_nr:
                nc.gpsimd.normalize_recip(out_ap=pr, in_ap=e2, denom_ap=sm)
            else:
                rc = outp.tile([CH, 1], F32, tag="rc", name="rc")
                nc.vector.reciprocal(out=rc, in_=sm)
                nc.vector.tensor_scalar_mul(out=pr, in0=e2, scalar1=rc)
        nc.sync.dma_start(out=probs[b_, c * CH : (c + 1) * CH, :], in_=pr)

    # ---- prologue: batch 0/1 projections ----
    kq0 = emit_kq_proj(iot0)
    va0 = emit_v_proj(iot0)

    BS = {}  # batch states
    kq_next = {0: kq0}
    va_next = {0: va0}
    iots = {0: iot0}
    if iot1 is not None:
        iots[1] = iot1

    def new_batch_state(b):
        usa = work.tile([E, N], FP16, tag="usa", name="usa")
        usb = work.tile([E, N], FP16, tag="usb", name="usb")
        et = [
            etp.tile([CH, H * COLW], FP16, tag=f"et{mc}", name=f"et{mc}")
            for mc in range(NCH)
        ]
        return {
            "b": b,
            "usa": usa,
            "usb": usb,
            "et": et,
            "kq": kq_next[b],
            "va": va_next[b],
            "iot": iots[b],
            "paT": {},
            "pbT": {},
        }

    def emit_st(bs, mc, h0, h1):
        stt = stp.tile([128, COLW * 3], F32, tag="st", name="stt")
        kq = bs["kq"]
        for i, h in enumerate(range(h0, h1)):
            kX = kq["kTa"] if h < 4 else kq["kTb"]
            qX = kq["qTa"] if h < 4 else kq["qTb"]
            hl = h % 4
            nc.tensor.matmul(
                stt[0:CH, i * COLW : i * COLW + N],
                kX[32 * hl : 32 * hl + D, mc * CH : (mc + 1) * CH],
                qX[32 * hl : 32 * hl + D, :],
                start=True,
                stop=True,
                tile_position=(32 * hl, 0),
            )
        return stt

    def emit_mask_exp(bs, stt, mc, h0, h1):
        nh = h1 - h0
        if with_mask:
            for i in range(nh):
                nc.vector.tensor_add(
                    out=stt[0:CH, i * COLW : i * COLW + N],
                    in0=stt[0:CH, i * COLW : i * COLW + N],
                    in1=bs["iot"]["mT"][mc],
                )
        etv = bs["et"][mc].rearrange("p (i c) -> p i c", c=COLW)
        stv = stt[0:CH].rearrange("p (i c) -> p i c", c=COLW)
        nc.scalar.activation(
            out=etv[:, h0 : h0 + nh, 0:N],
            in_=stv[:, 0:nh, 0:N],
            func=AF.Exp,
        )

    def emit_av(bs, mc, h0, h1):
        paT, pbT = bs["paT"], bs["pbT"]
        va, et = bs["va"], bs["et"]
        usa, usb = bs["usa"], bs["usb"]
        if mc not in paT:
            paT[mc] = pap.tile([128, COLW], F32, tag="pa", name="paT")
            pbT[mc] = pbp.tile([128, COLW], F32, tag="pb", name="pbT")
        for h in range(h0, h1):
            pt = paT[mc] if h < 4 else pbT[mc]
            hl = h % 4
            nc.tensor.matmul(
                pt[hl * 32 : (hl + 1) * 32, 0:N],
                va[mc][:, h * 32 : (h + 1) * 32],
                et[mc][:, h * COLW : h * COLW + N],
                start=True,
                stop=True,
                tile_position=(0, 32 * hl),
            )
        with nc.allow_low_precision(reason="fp16 U accum"):
            if h1 == 4 or (h0 <= 3 < h1):  # bank-A heads complete for mc
                if mc == 0:
                    nc.vector.tensor_copy(out=usa, in_=paT[mc][:, 0:N])
                else:
                    nc.vector.tensor_add(out=usa, in0=usa, in1=paT[mc][:, 0:N])
            if h1 == H:  # bank-B heads complete for mc
                if mc == 0:
                    nc.vector.tensor_copy(out=usb, in_=pbT[mc][:, 0:N])
                else:
                    nc.vector.tensor_add(out=usb, in0=usb, in1=pbT[mc][:, 0:N])

    # ---- global wave pipeline: per iteration [exp(gi-1), ST(gi), AV(gi-2)],
    # hooks only at wjs where the pa/pb accumulation windows are closed ----
    wave_seq = [
        (b, mc, h0, h1)
        for b in range(bpc)
        for mc in range(NCH)
        for (h0, h1) in WAVES
    ]
    rec = {}  # global index -> (bs, stt, mc, h0, h1)
    for gi, (b, mc, h0, h1) in enumerate(wave_seq):
        wj = gi % WPB
        if wj == 0:
            BS[b] = new_batch_state(b)
        bs = BS[b]
        # exp of previous wave first (keeps stt-ring WAR ordering sound)
        if gi >= 1:
            pbs, pstt, pmc, ph0, ph1 = rec[gi - 1]
            emit_mask_exp(pbs, pstt, pmc, ph0, ph1)
        rec[gi] = (bs, emit_st(bs, mc, h0, h1), mc, h0, h1)
        if gi >= 2:
            abs_, _astt, amc, ah0, ah1 = rec.pop(gi - 2)
            emit_av(abs_, amc, ah0, ah1)
        # hooks (after exp/ST/AV of this iteration)
        if wj == 0 and b + 2 <= bpc - 1:
            iots[b + 2] = emit_dma_in(b + 2)
        elif wj == 1 and b >= 1:
            emit_norm(BS[b - 1])
        elif wj == 4 and b >= 1:
            emit_final_chunk(BS[b - 1], 0)
        elif wj == 7:
            if b >= 1:
                emit_final_chunk(BS[b - 1], 1)
            if b + 1 <= bpc - 1:
                kq_next[b + 1] = emit_kq_proj(iots[b + 1])
                va_next[b + 1] = emit_v_proj(iots[b + 1])
        elif wj == 10 and b >= 1:
            emit_final_chunk(BS[b - 1], 2)
            emit_final_chunk(BS[b - 1], 3)

    # tail flush
    gi = len(wave_seq)
    pbs, pstt, pmc, ph0, ph1 = rec[gi - 1]
    emit_mask_exp(pbs, pstt, pmc, ph0, ph1)
    for k in (gi - 2, gi - 1):
        abs_, _astt, amc, ah0, ah1 = rec.pop(k)
        emit_av(abs_, amc, ah0, ah1)
    # Tail norm pipelined by 250-column halves: chunk 0/1's pointer work
    # starts as soon as the first half of mh exists, overlapping the second
    # half's normalize chain on DVE/PE with ACT's tanh/exp of half one.
    bs = BS[bpc - 1]
    usa, usb = bs["usa"], bs["usb"]
    HW_ = N // 2
    for half in range(2):
        cl, chh = half * HW_, (half + 1) * HW_
        rsA = pap.tile([128, COLW], F32, tag="pa", name="rsAh")
        rsB = pbp.tile([128, COLW], F32, tag="pb", name="rsBh")
        nc.tensor.matmul(
            rsA[:, 0:HW_], w["MEXP"], usa[:, cl:chh], start=True, stop=True
        )
        nc.tensor.matmul(
            rsB[:, 0:HW_], w["MEXP"], usb[:, cl:chh], start=True, stop=True
        )
        recA = work.tile([E, HW_], F32, tag="recAh", name="recAh")
        recB = work.tile([E, HW_], F32, tag="recBh", name="recBh")
        mhh = work.tile([E, HW_], FP16, tag="mhh", name="mhh")
        with nc.allow_low_precision(reason="fp16 softmax normalize"):
            nc.vector.reciprocal_approx_fast(out=recA, in_=rsA[:, 0:HW_])
            nc.vector.reciprocal_approx_fast(out=recB, in_=rsB[:, 0:HW_])
            ota = work.tile([E, HW_], FP16, tag="otah", name="otah")
            otb = work.tile([E, HW_], FP16, tag="otbh", name="otbh")
            nc.vector.tensor_mul(out=ota, in0=usa[:, cl:chh], in1=recA)
            nc.vector.tensor_mul(out=otb, in0=usb[:, cl:chh], in1=recB)
        mps = pap.tile([128, COLW], F32, tag="pa", name="mpsh")
        nc.tensor.matmul(mps[:, 0:HW_], w["WcA"], ota, start=True, stop=False)
        nc.tensor.matmul(mps[:, 0:HW_], w["WcB"], otb, start=False, stop=True)
        with nc.allow_low_precision(reason="fp16 mh for pointer matmul"):
            nc.vector.tensor_copy(out=mhh, in_=mps[:, 0:HW_])
        hbs = dict(bs)
        hbs["mh"] = mhh
        emit_final_chunk(hbs, 2 * half, mh_c=0)
        emit_final_chunk(hbs, 2 * half + 1, mh_c=1)


def build(bpc=BPC, with_mask=False, use_pool_nr=True):
    nc = bacc.Bacc("TRN2", target_bir_lowering=False, debug=False)
    shapes = {
        "xfT": (bpc, E, N),
        "q1T": (bpc, E, N),
        "q0T": (bpc, E, N),
        "Wq1A": (E, E),
        "Wq1B": (E, E),
        "Wq0A": (E, E),
        "Wq0B": (E, E),
        "WkA": (E, E),
        "WkB": (E, E),
        "Wv": (E, E),
        "WcA": (E, E),
        "WcB": (E, E),
        "MEXP": (E, E),
    }
    if with_mask:
        shapes["maskT"] = (bpc, N, N)
        shapes["maskN"] = (bpc, N, N)
    f32_names = {"maskT", "maskN"}
    ap = {
        k: nc.dram_tensor(
            k, list(s), F32 if k in f32_names else FP16, kind="ExternalInput"
        ).ap()
        for k, s in shapes.items()
    }
    probs = nc.dram_tensor("probs", [bpc, N, N], FP16, kind="ExternalOutput").ap()
    with tile.TileContext(nc) as tc:
        with ExitStack() as ctx:
            _emit(ctx, tc, ap, probs, bpc, with_mask, use_pool_nr)
    nc.compile()
    return nc


def _pad_cols(W, half):
    """[E, 64] head-cols of `half` spread to [E, 128] at 32-col boundaries."""
    out = np.zeros((E, E), np.float16)
    for hl in range(4):
        h = half * 4 + hl
        out[:, 32 * hl : 32 * hl + D] = W[:, h * D : (h + 1) * D]
    return out


def host_constants(Wq1, Wq0, Wk, Wv, Wc, bc):
    Wq1 = np.asarray(Wq1, np.float32)
    Wq0 = np.asarray(Wq0, np.float32)
    Wks = np.asarray(Wk, np.float32) * 0.25
    Wc = np.asarray(Wc, np.float32)
    wca = np.zeros((E, E), np.float32)
    wcb = np.zeros((E, E), np.float32)
    for hl in range(4):
        wca[32 * hl : 32 * hl + D, :] = Wc[hl * D : (hl + 1) * D, :]
        wcb[32 * hl : 32 * hl + D, :] = Wc[(hl + 4) * D : (hl + 5) * D, :]
    # bias folded into the ota==1 row (band-0 rowsum row normalizes to 1)
    wca[16, :] = np.asarray(bc, np.float32)
    mexp = np.zeros((E, E), np.float16)
    for p in range(E):
        mexp[32 * (p // 32) + 16, p] = 1.0
    return {
        "Wq1A": _pad_cols(Wq1, 0),
        "Wq1B": _pad_cols(Wq1, 1),
        "Wq0A": _pad_cols(Wq0, 0),
        "Wq0B": _pad_cols(Wq0, 1),
        "WkA": _pad_cols(Wks, 0),
        "WkB": _pad_cols(Wks, 1),
        "Wv": np.asarray(Wv, np.float16),
        "WcA": wca.astype(np.float16),
        "WcB": wcb.astype(np.float16),
        "MEXP": mexp,
    }


def host_in_map(inputs, c, bpc=BPC, with_mask=False):
    """Per-core input dict for core c (batches c*bpc .. (c+1)*bpc)."""
    sl = slice(c * bpc, (c + 1) * bpc)
    x = np.asarray(inputs["encoded_nodes_f"], np.float32)[sl]
    q1 = np.asarray(inputs["encoded_q1_t"], np.float32)[sl]
    q0 = np.asarray(inputs["encoded_q0"], np.float32)[sl]
    m = {
        "xfT": np.ascontiguousarray(x.transpose(0, 2, 1)).astype(np.float16),
        "q1T": np.ascontiguousarray(q1.transpose(0, 2, 1)).astype(np.float16),
        "q0T": np.ascontiguousarray(q0.transpose(0, 2, 1)).astype(np.float16),
    }
    m.update(
        host_constants(
            inputs["Wq1"],
            inputs["Wq0"],
            inputs["Wk"],
            inputs["Wv"],
            inputs["Wc"],
            inputs["bc"],
        )
    )
    if with_mask:
        mask = np.asarray(inputs["ninf_mask"], np.float32)[sl]
        m["maskT"] = np.ascontiguousarray(mask.transpose(0, 2, 1))
        m["maskN"] = np.ascontiguousarray(mask)
    return m


_NC_CACHE = {}


def _get_nc(with_mask):
    if with_mask not in _NC_CACHE:
        _NC_CACHE[with_mask] = build(BPC, with_mask)
    return _NC_CACHE[with_mask]


def _ensure_ntff_hook():
    """Register the axon NTFF profile hook if the image's antenv lacks it."""
    import types

    try:
        from antenv.axon_hooks import get_axon_ntff_profile_hook  # noqa: F401

        return
    except ImportError:
        pass
    import antenv

    mod = types.ModuleType("antenv.axon_hooks")
    _h = {}
    mod.set_axon_ntff_profile_hook = lambda hook: _h.__setitem__("h", hook)
    mod.get_axon_ntff_profile_hook = lambda: _h.get("h")
    sys.modules["antenv.axon_hooks"] = mod
    antenv.axon_hooks = mod
    try:
        if "/root/.axon_site/trn_agent_boot" not in sys.path:
            sys.path.insert(0, "/root/.axon_site/trn_agent_boot")
        from trn_boot import _ntff_profile_via_ctypes

        mod.set_axon_ntff_profile_hook(
            _ntff_profile_via_ctypes("/opt/axon/libaxon_pjrt.so")
        )
    except Exception as e:  # degrade to no-trace
        print("ntff hook registration failed:", e)


def run(inputs, trace=False):
    """Run on 8 cores; returns (full probs array, BassKernelResults)."""
    from concourse.bass_utils import run_bass_kernel_spmd

    if trace:
        _ensure_ntff_hook()

    with_mask = bool(np.any(np.asarray(inputs["ninf_mask"])))
    nc = _get_nc(with_mask)
    in_maps = [host_in_map(inputs, c, BPC, with_mask) for c in range(NCORES)]
    res = run_bass_kernel_spmd(nc, in_maps, list(range(NCORES)), trace=trace)
    out = np.empty((B, N, N), np.float32)
    for c in range(NCORES):
        out[c * BPC : (c + 1) * BPC] = res.results[c]["probs"].astype(np.float32)
    return out, res


def kernel(**inputs):
    out, _ = run(inputs)
    return out
